# revision 2
# baseline (speedup 1.0000x reference)
"""Trainium2 Bass kernel for a dense transformer block (nn_Block_86595130622141).

Reference computation (B=1, T=4096, C=1024, H=16 heads, hd=64, FFN=4C):
    h   = LN(x, g1, be1)
    q,k,v = per-head projections of h         (Wq/Wk/Wv: [H, C, hd])
    attn  = causal softmax(q k^T / sqrt(C)) v  (per head)
    x2  = x + concat(attn) @ Wp + bp
    out = x2 + relu(LN(x2, g2, be2) @ W1 + b1) @ W2 + b2

Sharding over 8 NeuronCores:
  - attention (incl. q/k/v projections) is tensor-parallel over heads:
    core j owns heads {2j, 2j+1} over the full sequence,
  - projection/LN2/FFN/residuals are sequence-parallel: core j owns rows
    [512j, 512j+512),
  - two bf16 AllToAlls ship UNNORMALIZED attention outputs + per-row softmax
    sums ([65, 512] tiles); the destination core normalizes during the
    projection phase (reciprocal + fp32r rank-2 broadcast matmul).

Performance design (measured on HW):
  - PE runs N=512 bf16 matmuls at ~269ns ONLY when the Scalar activation
    stream per pipeline period stays below the PE stream; exp is therefore
    issued on PAIRED [128,1024] psum tiles (one activation per two score
    matmuls), with adjacent matmuls sharing lhsT so LDWEIGHTS is skipped.
  - attention inner loop: scores(i) -> exp(i) -> AV(i-2)  (lookahead 2).
  - diagonal tiles: exp only over the live column range; dead columns
    memset to 0; the triangular wedge is masked by one [128,128] 0/1
    multiply on Vector (bf16 2x mode).
  - residual/bias injection never uses fp32 identity matmuls: residuals are
    added by Vector during PSUM evacuation; biases enter PSUM via K=1 bf16
    matmuls.
  - LN gammas/betas are folded into adjacent projection weights on the host.
"""

import numpy as np
import ml_dtypes

import concourse.bass as bass
import concourse.mybir as mybir
import concourse.tile as tile
from concourse import bacc
from concourse.bass_utils import run_bass_kernel_spmd
from concourse.masks import make_identity

F32 = mybir.dt.float32
F32R = mybir.dt.float32r
BF16 = mybir.dt.bfloat16
AF = mybir.ActivationFunctionType
ALU = mybir.AluOpType

N_CORES = 8
T = 4096
C = 1024
H = 16
HD = 64
FF = 4096
LN_EPS = 1e-5
ROWS = T // N_CORES          # 512 rows per core (sequence shard)
HPC = H // N_CORES           # 2 heads per core
D2 = HPC * HD                # 128: stacked head dim per core
NCT = C // 128               # 8 c-tiles
NTB = T // 512               # 8 t-blocks of 512
NFB = FF // 128              # 32 f-tiles
SCALE = float(C) ** -0.5


def _bf16(a):
    return np.ascontiguousarray(np.asarray(a, dtype=np.float32)).astype(ml_dtypes.bfloat16)


def _f32(a):
    return np.ascontiguousarray(np.asarray(a, dtype=np.float32))


def build_program():
    nc = bacc.Bacc("TRN2", target_bir_lowering=False, debug=False,
                   num_devices=N_CORES)

    dt = nc.dram_tensor
    d = {
        "xb": dt("xb", [T, C], BF16, kind="ExternalInput").ap(),
        "xr": dt("xr", [ROWS, C], F32, kind="ExternalInput").ap(),
        "wq": dt("wq", [NCT, 128, D2], BF16, kind="ExternalInput").ap(),
        "wk": dt("wk", [NCT, 128, D2], BF16, kind="ExternalInput").ap(),
        "wv": dt("wv", [NCT, 128, D2], BF16, kind="ExternalInput").ap(),
        "bq": dt("bq", [D2], F32, kind="ExternalInput").ap(),
        "bk": dt("bk", [D2], F32, kind="ExternalInput").ap(),
        "bv": dt("bv", [D2], BF16, kind="ExternalInput").ap(),
        "wp": dt("wp", [NCT, 128, C], BF16, kind="ExternalInput").ap(),
        "bp": dt("bp", [C], BF16, kind="ExternalInput").ap(),
        "w1": dt("w1", [NFB, 128, NCT, 128], BF16, kind="ExternalInput").ap(),
        "b1": dt("b1", [128, NFB], F32, kind="ExternalInput").ap(),
        "w2": dt("w2", [NFB, 128, C], BF16, kind="ExternalInput").ap(),
        "b2": dt("b2", [C], BF16, kind="ExternalInput").ap(),
        "cmw": dt("cmw", [128, 128], BF16, kind="ExternalInput").ap(),
        "e16": dt("e16", [8, NCT, 64], BF16, kind="ExternalInput").ap(),
        "out": dt("out", [ROWS, C], F32, kind="ExternalOutput").ap(),
        "a2ai0": dt("a2a_in0", [N_CORES, 65, 512], BF16).ap(),
        "a2ao0": dt("a2a_out0", [N_CORES, 65, 512], BF16).ap(),
        "a2ai1": dt("a2a_in1", [N_CORES, 65, 512], BF16).ap(),
        "a2ao1": dt("a2a_out1", [N_CORES, 65, 512], BF16).ap(),
    }

    with tile.TileContext(nc) as tc:
        _build(nc, tc, d)
    nc.compile()
    return nc


def _build(nc, tc, d):
    from contextlib import ExitStack

    stack = ExitStack()
    with stack:
        const = stack.enter_context(tc.tile_pool(name="const", bufs=1))

        # ---------- constants ----------
        ident = const.tile([128, 128], BF16)
        make_identity(nc, ident)
        eps_t = const.tile([128, 1], F32)
        nc.vector.memset(eps_t, LN_EPS)
        ones1 = const.tile([1, 128], BF16)
        nc.vector.memset(ones1, 1.0)
        ones_row = const.tile([1, 512], BF16)
        nc.vector.memset(ones_row, 1.0)

        cmw_sb = const.tile([128, 128], BF16)
        nc.sync.dma_start(out=cmw_sb, in_=d["cmw"])
        e8_sb = const.tile([8, NCT, 64], BF16)
        nc.sync.dma_start(out=e8_sb, in_=d["e16"])

        bq_sb = const.tile([128, 1], F32)
        nc.sync.dma_start(out=bq_sb, in_=d["bq"][:])
        bk_sb = const.tile([128, 1], F32)
        nc.sync.dma_start(out=bk_sb, in_=d["bk"][:])
        bp_row = const.tile([1, C], BF16)
        nc.sync.dma_start(out=bp_row, in_=d["bp"][:])
        b2_row = const.tile([1, C], BF16)
        nc.sync.dma_start(out=b2_row, in_=d["b2"][:])
        b1_sb = const.tile([128, NFB], F32)
        nc.sync.dma_start(out=b1_sb, in_=d["b1"])

        # bv broadcast across t-partitions: [t_part, d2] via K=1 matmul
        bv_bc = const.tile([128, D2], F32)
        with tc.tile_pool(name="bcast_ps", bufs=1, space="PSUM") as bcast_ps:
            bv_row = const.tile([1, D2], BF16)
            nc.sync.dma_start(out=bv_row, in_=d["bv"][:])
            ps_bv = bcast_ps.tile([128, D2], F32, tag="bc")
            nc.tensor.matmul(ps_bv, lhsT=ones1, rhs=bv_row, start=True, stop=True)
            nc.vector.tensor_copy(out=bv_bc, in_=ps_bv)

        # ---------- attention-phase persistent tensors ----------
        attn_stack = stack.enter_context(ExitStack())
        attn_pool = attn_stack.enter_context(tc.tile_pool(name="attn_p", bufs=1))
        qT = [attn_pool.tile([128, 512], BF16, tag=f"qT{b}", name=f"qT{b}")
              for b in range(NTB)]
        kT = [attn_pool.tile([128, 512], BF16, tag=f"kT{b}", name=f"kT{b}")
              for b in range(NTB)]
        # v with a trailing ones column per head (softmax sums land on PSUM
        # partition 64): [tk_r, tk_tile_in_block, head, 64+1]
        vv = [attn_pool.tile([128, 4, HPC, 65], BF16, tag=f"v{b}", name=f"v{b}")
              for b in range(NTB)]
        for b in range(NTB):
            nc.vector.memset(vv[b][:, :, :, 64:65], 1.0)

        # ---------- phase A: LN1 + transpose + QKV, per t-block ----------
        with nc.named_scope("phA"):
            with (
                tc.tile_pool(name="h1T_p", bufs=1) as h1T_p,
                tc.tile_pool(name="ln_x", bufs=6) as ln_x,
                tc.tile_pool(name="ln_tmp", bufs=6) as ln_tmp,
                tc.tile_pool(name="tr_ps", bufs=2, space="PSUM") as tr_ps,
                tc.tile_pool(name="qkv_ps", bufs=2, space="PSUM") as qkv_ps,
                tc.tile_pool(name="wqkv", bufs=1) as wqkv,
            ):
                wq_sb = wqkv.tile([128, NCT, D2], BF16)
                wk_sb = wqkv.tile([128, NCT, D2], BF16)
                wv_sb = wqkv.tile([128, NCT, D2], BF16)
                xb0 = []
                for tsub in range(4):
                    x_t = ln_x.tile([128, C], BF16, tag="x", name=f"x0{tsub}")
                    nc.sync.dma_start(out=x_t,
                                      in_=d["xb"][128 * tsub:128 * tsub + 128])
                    xb0.append(x_t)
                for w_sb, key in ((wq_sb, "wq"), (wk_sb, "wk"), (wv_sb, "wv")):
                    nc.sync.dma_start(
                        out=w_sb, in_=d[key].rearrange("c p d -> p c d"))

                h1T = [h1T_p.tile([128, NCT, 512], BF16, tag=f"h1T{b}",
                                  name=f"h1T{b}")
                       for b in range(NTB)]

                def ln_transpose(tb):
                    for tsub in range(4):
                        ti = 4 * tb + tsub
                        if tb == 0:
                            x_t = xb0[tsub]
                        else:
                            x_t = ln_x.tile([128, C], BF16, tag="x",
                                            name=f"x{tb}{tsub}")
                            nc.sync.dma_start(
                                out=x_t, in_=d["xb"][128 * ti:128 * ti + 128])
                        stats = ln_tmp.tile([128, 2, 6], F32, tag="stats")
                        xr2 = x_t.rearrange("p (s d) -> p s d", s=2)
                        for sg in range(2):
                            nc.vector.bn_stats(out=stats[:, sg, :], in_=xr2[:, sg, :])
                        mv = ln_tmp.tile([128, 2], F32, tag="mv")
                        nc.vector.bn_aggr(out=mv, in_=stats)
                        rstd = ln_tmp.tile([128, 1], F32, tag="rstd")
                        nc.scalar.activation(out=rstd, in_=mv[:, 1:2], func=AF.Sqrt,
                                             bias=eps_t, scale=1.0)
                        nc.vector.reciprocal(out=rstd, in_=rstd)
                        h1c = ln_tmp.tile([128, C], BF16, tag="h1c")
                        nc.vector.tensor_scalar(out=h1c, in0=x_t,
                                                scalar1=mv[:, 0:1],
                                                scalar2=rstd, op0=ALU.subtract,
                                                op1=ALU.mult)
                        # 8 transposes into ONE [128,1024]bf16 psum bank,
                        # single evacuation copy (Scalar) per x-tile
                        ps_t = tr_ps.tile([128, 8, 128], BF16, tag="tr")
                        for ci in range(NCT):
                            nc.tensor.transpose(
                                ps_t[:, ci, :], h1c[:, 128 * ci:128 * ci + 128],
                                ident)
                        nc.scalar.activation(
                            out=h1T[tb][:, :, 128 * tsub:128 * tsub + 128],
                            in_=ps_t, func=AF.Copy, scale=1.0)

                def qkv(tb):
                    # q^T / k^T for this t-block: out[d2=128, t=512]
                    ps_q = qkv_ps.tile([128, 512], F32, tag="q", name=f"q{tb}")
                    ps_k = qkv_ps.tile([128, 512], F32, tag="k", name=f"k{tb}")
                    for ci in range(NCT):
                        nc.tensor.matmul(ps_q, lhsT=wq_sb[:, ci, :],
                                         rhs=h1T[tb][:, ci, :],
                                         start=(ci == 0), stop=(ci == NCT - 1))
                    for ci in range(NCT):
                        nc.tensor.matmul(ps_k, lhsT=wk_sb[:, ci, :],
                                         rhs=h1T[tb][:, ci, :],
                                         start=(ci == 0), stop=(ci == NCT - 1))
                    nc.vector.tensor_scalar_add(out=qT[tb], in0=ps_q, scalar1=bq_sb)
                    nc.vector.tensor_scalar_add(out=kT[tb], in0=ps_k, scalar1=bk_sb)
                    # v natural: out[t=128, d2], lhsT = h1T tile [c_tile, t_tile]
                    for tsub in range(4):
                        ps_v = qkv_ps.tile([128, D2], F32, tag="v",
                                           name=f"v{tb}{tsub}")
                        for ci in range(NCT):
                            nc.tensor.matmul(
                                ps_v,
                                lhsT=h1T[tb][:, ci, 128 * tsub:128 * tsub + 128],
                                rhs=wv_sb[:, ci, :],
                                start=(ci == 0), stop=(ci == NCT - 1))
                        nc.vector.tensor_tensor(
                            out=vv[tb][:, tsub, :, 0:64],
                            in0=ps_v.rearrange("p (h d) -> p h d", h=HPC),
                            in1=bv_bc.rearrange("p (h d) -> p h d", h=HPC),
                            op=ALU.add)

                # qkv(tb-1) is emitted after transposes(tb) so the qk matmuls
                # never wait on the just-finished Scalar evacuation
                for tb in range(NTB):
                    ln_transpose(tb)
                    if tb > 0:
                        qkv(tb - 1)
                qkv(NTB - 1)

        # ---------- phase B: attention ----------
        # Two "workers" (psum pair halves) each process a fixed sequence of
        # t-blocks chosen so both finish together (32+20+16+4 = 28+24+12+8
        # = 72 key-tile steps): the PE stream never thins, which keeps the
        # Tensor engine clock at full speed. One paired exp per period.
        a2a_io = [(d["a2ai0"], d["a2ao0"]), (d["a2ai1"], d["a2ao1"])]
        SEQ_A = [7, 4, 3, 0]
        SEQ_B = [6, 5, 2, 1]
        steps_w = [[(tb, i) for tb in seq for i in range(4 * (tb + 1))]
                   for seq in (SEQ_A, SEQ_B)]
        NP = len(steps_w[0])
        assert len(steps_w[1]) == NP
        with nc.named_scope("phB"):
            with (
                tc.tile_pool(name="at_sp", bufs=3, space="PSUM") as sp_p,
                tc.tile_pool(name="at_pa", bufs=1, space="PSUM") as pa_p,
                tc.tile_pool(name="at_w", bufs=4) as w_p,
                tc.tile_pool(name="at_fin", bufs=4) as fin_p,
            ):
                # both heads fused into one continuous stream: period p
                # handles head p//NP, worker step p%NP — the PE never idles
                # at the head boundary; each A2A fires as soon as its head's
                # last block is staged
                ps_a = [None, None]
                hist = {}
                left = [len(SEQ_A) + len(SEQ_B)] * HPC
                for p in range(2 * NP + 2):
                    if p < 2 * NP:
                        h = p // NP
                        hs = 64 * h
                        pp = p % NP
                        pair = sp_p.tile([128, 1024], F32, tag="sp",
                                         name=f"sp{p}")
                        wt = w_p.tile([128, 1024], BF16, tag="w",
                                      name=f"w{p}")
                        los = [0, 0]
                        for s in range(2):
                            tb, i = steps_w[s][pp]
                            tkb, tks = i // 4, i % 4
                            kt = kT[tkb][hs:hs + 64,
                                         128 * tks:128 * tks + 128]
                            nc.tensor.matmul(
                                pair[:, 512 * s:512 * s + 512], lhsT=kt,
                                rhs=qT[tb][hs:hs + 64, :],
                                start=True, stop=True)
                            r = i - 4 * tb
                            if 0 <= r < 4:
                                los[s] = 128 * r
                        if los[0] == 0 and los[1] == 0:
                            nc.scalar.activation(out=wt, in_=pair,
                                                 func=AF.Exp, scale=SCALE)
                        elif los[1] == 0:
                            nc.vector.memset(wt[:, 0:los[0]], 0.0)
                            nc.scalar.activation(out=wt[:, los[0]:1024],
                                                 in_=pair[:, los[0]:1024],
                                                 func=AF.Exp, scale=SCALE)
                        else:
                            if los[0] > 0:
                                nc.vector.memset(wt[:, 0:los[0]], 0.0)
                            nc.scalar.activation(out=wt[:, los[0]:512],
                                                 in_=pair[:, los[0]:512],
                                                 func=AF.Exp, scale=SCALE)
                            nc.vector.memset(wt[:, 512:512 + los[1]], 0.0)
                            nc.scalar.activation(
                                out=wt[:, 512 + los[1]:1024],
                                in_=pair[:, 512 + los[1]:1024],
                                func=AF.Exp, scale=SCALE)
                        for s in range(2):
                            tb, i = steps_w[s][pp]
                            if i >= 4 * tb:
                                lo = 512 * s + 128 * (i - 4 * tb)
                                nc.vector.tensor_tensor(
                                    out=wt[:, lo:lo + 128],
                                    in0=wt[:, lo:lo + 128],
                                    in1=cmw_sb, op=ALU.mult)
                        hist[p] = wt
                    q = p - 2
                    if q >= 0 and q in hist:
                        wtq = hist.pop(q)
                        hq = q // NP
                        qq = q % NP
                        for s in range(2):
                            tb, i = steps_w[s][qq]
                            tkb, tks = i // 4, i % 4
                            vt = vv[tkb][:, tks, hq, :]
                            if i == 0:
                                ps_a[s] = pa_p.tile(
                                    [65, 512], F32, tag=f"a{s}",
                                    name=f"pa{hq}{s}{tb}")
                            nc.tensor.matmul(
                                ps_a[s], lhsT=vt,
                                rhs=wtq[:, 512 * s:512 * s + 512],
                                start=(i == 0),
                                stop=(i == 4 * (tb + 1) - 1))
                            if i == 4 * (tb + 1) - 1:
                                at_sb = fin_p.tile([65, 512], BF16,
                                                   tag="fin",
                                                   name=f"fin{hq}{tb}")
                                nc.vector.tensor_copy(out=at_sb,
                                                      in_=ps_a[s])
                                nc.sync.dma_start(out=a2a_io[hq][0][tb],
                                                  in_=at_sb)
                                left[hq] -= 1
                                if left[hq] == 0:
                                    nc.gpsimd.collective_compute(
                                        "AllToAll", ALU.bypass,
                                        replica_groups=[list(range(N_CORES))],
                                        ins=[a2a_io[hq][0][:]],
                                        outs=[a2a_io[hq][1][:]])

        attn_stack.close()

        # ---------- phase C: normalize + projection + LN2 + transpose ------
        late = stack.enter_context(tc.tile_pool(name="late", bufs=1))
        x_rows = late.tile([128, 4, C], F32)
        for tt in range(4):
            nc.sync.dma_start(out=x_rows[:, tt, :],
                              in_=d["xr"][128 * tt:128 * tt + 128])
        x2_sb = late.tile([128, 4, C], F32)
        h2T = late.tile([128, NCT, 512], BF16)
        with nc.named_scope("phC"):
            with (
                tc.tile_pool(name="pr_g", bufs=1) as g_p,
                tc.tile_pool(name="pr_wp", bufs=1) as wp_p,
                tc.tile_pool(name="pr_ps", bufs=4, space="PSUM") as pr_ps,
                tc.tile_pool(name="pr_bc", bufs=2, space="PSUM") as bc_ps,
                tc.tile_pool(name="pr_tmp", bufs=4) as pr_tmp,
                tc.tile_pool(name="tr2_ps", bufs=2, space="PSUM") as tr2_ps,
            ):
                wp_sb = wp_p.tile([128, NCT, C], BF16)
                nc.sync.dma_start(
                    out=wp_sb, in_=d["wp"].rearrange("c p d -> p c d"))
                # all of w1 is loaded up-front, and its DMAs are issued
                # BEFORE the A2A1-gated gat loads so the Sync queue never
                # blocks the FFN weight stream
                w1_all = late.tile([128, NFB, NCT, 128], BF16)
                nc.sync.dma_start(
                    out=w1_all, in_=d["w1"].rearrange("f p c d -> p f c d"))

                gat = g_p.tile([128, NCT, 512], BF16)
                # normalize each half as soon as its A2A lands: half hx=0
                # (heads 2ci, rows 0:64, from a2a0) can run during phase B
                for hx, a2ao, po in ((0, d["a2ao0"], 0), (1, d["a2ao1"], 64)):
                    nc.sync.dma_start(
                        out=gat[po:po + 64, :, :],
                        in_=a2ao[:, 0:64, :].rearrange("c p d -> p c d"))
                    sums_h = g_p.tile([8, 512], BF16, name=f"sums{hx}")
                    nc.sync.dma_start(out=sums_h, in_=a2ao[:, 64, :])
                    recip_h = g_p.tile([8, 512], F32, name=f"recip{hx}")
                    nc.vector.reciprocal(out=recip_h, in_=sums_h)
                    recip_bf = g_p.tile([8, 512], BF16, name=f"recipb{hx}")
                    nc.vector.tensor_copy(out=recip_bf, in_=recip_h)
                    for ci in range(NCT):
                        ps_bc = bc_ps.tile([64, 512], F32, tag="bc",
                                           name=f"bc{hx}{ci}")
                        nc.tensor.matmul(
                            ps_bc,
                            lhsT=e8_sb[:, ci, :],
                            rhs=recip_bf,
                            start=True, stop=True)
                        nc.vector.tensor_tensor(out=gat[po:po + 64, ci, :],
                                                in0=gat[po:po + 64, ci, :],
                                                in1=ps_bc,
                                                op=ALU.mult)

                # projection in two waves of 4 psums: the even-head
                # (rows 0:64) half-contraction runs BEFORE the second A2A
                # lands, filling the collective wait with PE work
                def proj_full(tt, nbk):
                    ns = slice(512 * nbk, 512 * nbk + 512)
                    ps_p = pr_ps.tile([128, 512], F32, tag="pp",
                                      name=f"pp{tt}{nbk}")
                    nc.tensor.matmul(ps_p, lhsT=ones1, rhs=bp_row[:, ns],
                                     start=True, stop=False)
                    for ci in range(NCT):
                        nc.tensor.matmul(
                            ps_p,
                            lhsT=gat[:, ci, 128 * tt:128 * tt + 128],
                            rhs=wp_sb[:, ci, ns],
                            start=False, stop=(ci == NCT - 1))
                    nc.vector.tensor_tensor(out=x2_sb[:, tt, ns],
                                            in0=ps_p,
                                            in1=x_rows[:, tt, ns],
                                            op=ALU.add)

                def ln2(tt):
                    # LN2 for this row-tile
                    stats = pr_tmp.tile([128, 2, 6], F32, tag="stats")
                    x2r = x2_sb[:, tt, :].rearrange("p (s d) -> p s d", s=2)
                    for sg in range(2):
                        nc.vector.bn_stats(out=stats[:, sg, :], in_=x2r[:, sg, :])
                    mv = pr_tmp.tile([128, 2], F32, tag="mv")
                    nc.vector.bn_aggr(out=mv, in_=stats)
                    rstd = pr_tmp.tile([128, 1], F32, tag="rstd")
                    nc.scalar.activation(out=rstd, in_=mv[:, 1:2], func=AF.Sqrt,
                                         bias=eps_t, scale=1.0)
                    nc.vector.reciprocal(out=rstd, in_=rstd)
                    h2c = pr_tmp.tile([128, C], BF16, tag="h2c")
                    nc.vector.tensor_scalar(out=h2c, in0=x2_sb[:, tt, :],
                                            scalar1=mv[:, 0:1], scalar2=rstd,
                                            op0=ALU.subtract, op1=ALU.mult)
                    ps_t = tr2_ps.tile([128, 8, 128], BF16, tag="tr")
                    for ci in range(NCT):
                        nc.tensor.transpose(
                            ps_t[:, ci, :], h2c[:, 128 * ci:128 * ci + 128],
                            ident)
                    nc.scalar.activation(
                        out=h2T[:, :, 128 * tt:128 * tt + 128],
                        in_=ps_t, func=AF.Copy, scale=1.0)

                for tt in range(4):
                    for nbk in range(2):
                        proj_full(tt, nbk)
                    ln2(tt)

        # ---------- phase D: FFN ----------
        with nc.named_scope("phD"):
            ff_sb = late.tile([128, NFB, 512], BF16)
            with (
                tc.tile_pool(name="f1_ps", bufs=4, space="PSUM") as f1_ps,
            ):
                for fb in range(NFB):
                    ps_f = f1_ps.tile([128, 512], F32, tag="pf")
                    for ci in range(NCT):
                        nc.tensor.matmul(ps_f, lhsT=w1_all[:, fb, ci, :],
                                         rhs=h2T[:, ci, :],
                                         start=(ci == 0), stop=(ci == NCT - 1))
                    nc.scalar.activation(out=ff_sb[:, fb, :], in_=ps_f,
                                         func=AF.Relu,
                                         bias=b1_sb[:, fb:fb + 1], scale=1.0)

            with (
                tc.tile_pool(name="f2_w", bufs=6) as f2_w,
                tc.tile_pool(name="f2_ps", bufs=1, space="PSUM") as f2_ps,
                tc.tile_pool(name="out_sb", bufs=2) as out_p,
            ):
                ps_o = [[f2_ps.tile([128, 512], F32, tag=f"o{tt}{nbk}",
                                    name=f"o{tt}{nbk}")
                         for nbk in range(2)] for tt in range(4)]
                for tt in range(4):
                    for nbk in range(2):
                        ns = slice(512 * nbk, 512 * nbk + 512)
                        nc.tensor.matmul(ps_o[tt][nbk], lhsT=ones1,
                                         rhs=b2_row[:, ns], start=True,
                                         stop=False)
                for fb in range(NFB):
                    w2_sb = f2_w.tile([128, C], BF16, tag="w2")
                    nc.sync.dma_start(out=w2_sb, in_=d["w2"][fb])
                    for tt in range(4):
                        for nbk in range(2):
                            nc.tensor.matmul(
                                ps_o[tt][nbk],
                                lhsT=ff_sb[:, fb, 128 * tt:128 * tt + 128],
                                rhs=w2_sb[:, 512 * nbk:512 * nbk + 512],
                                start=False, stop=(fb == NFB - 1))
                for tt in range(4):
                    o_t = out_p.tile([128, C], F32, tag="o")
                    for nbk in range(2):
                        ns = slice(512 * nbk, 512 * nbk + 512)
                        nc.vector.tensor_tensor(out=o_t[:, ns],
                                                in0=ps_o[tt][nbk],
                                                in1=x2_sb[:, tt, ns],
                                                op=ALU.add)
                    nc.sync.dma_start(out=d["out"][128 * tt:128 * tt + 128],
                                      in_=o_t)


_NC_CACHE = None


def _get_program():
    global _NC_CACHE
    if _NC_CACHE is None:
        _NC_CACHE = build_program()
    return _NC_CACHE


def make_in_maps(inputs):
    x = _f32(inputs["x"]).reshape(T, C)
    Wq = _f32(inputs["Wq"])
    Wk = _f32(inputs["Wk"])
    Wv = _f32(inputs["Wv"])
    Wp = _f32(inputs["Wp"])
    bp = _f32(inputs["bp"])
    W1 = _f32(inputs["W1"])
    b1 = _f32(inputs["b1"])
    W2 = _f32(inputs["W2"])
    b2 = _f32(inputs["b2"])
    g1 = _f32(inputs["g1"])
    be1 = _f32(inputs["be1"])
    g2 = _f32(inputs["g2"])
    be2 = _f32(inputs["be2"])

    # fold LN affine params into adjacent projections
    Wq_f = g1[None, :, None] * Wq          # [H, C, hd]
    Wk_f = g1[None, :, None] * Wk
    Wv_f = g1[None, :, None] * Wv
    bq_f = np.einsum("c,hcd->hd", be1, Wq)  # [H, hd]
    bk_f = np.einsum("c,hcd->hd", be1, Wk)
    bv_f = np.einsum("c,hcd->hd", be1, Wv)
    W1_f = g2[:, None] * W1                 # [C, FF]
    b1_f = b1 + be2 @ W1                    # [FF]

    # causal wedge mask for diagonal 128x128 sub-tiles: key_local <= q_local
    tk_l = np.arange(128)[:, None]
    tq_l = np.arange(128)[None, :]
    cmw = (tk_l <= tq_l).astype(np.float32)

    # E8 selector per c-tile: lhsT [8, 64] broadcasting recip row ci of the
    # current half into all 64 output partitions
    e16 = np.zeros((8, NCT, 64), dtype=np.float32)
    for ci in range(NCT):
        e16[ci, ci, :] = 1.0

    w1_host = _bf16(W1_f.reshape(NCT, 128, NFB, 128).transpose(2, 1, 0, 3))
    b1_host = _f32(b1_f.reshape(NFB, 128).T)
    w2_host = _bf16(W2.reshape(NFB, 128, C))
    wp_host = _bf16(Wp.reshape(NCT, 128, C))
    cmw_host = _bf16(cmw)
    xb_host = _bf16(x)
    bp_host = _bf16(bp)
    b2_host = _bf16(b2)
    e16_host = _bf16(e16)

    in_maps = []
    for j in range(N_CORES):
        hsl = slice(HPC * j, HPC * j + HPC)
        wq_l = np.concatenate([Wq_f[h] for h in range(HPC * j, HPC * j + HPC)],
                              axis=1)
        wk_l = np.concatenate([Wk_f[h] for h in range(HPC * j, HPC * j + HPC)],
                              axis=1)
        wv_l = np.concatenate([Wv_f[h] for h in range(HPC * j, HPC * j + HPC)],
                              axis=1)
        in_maps.append({
            "xb": xb_host,
            "xr": _f32(x[ROWS * j:ROWS * j + ROWS]),
            "wq": _bf16(wq_l.reshape(NCT, 128, D2)),
            "wk": _bf16(wk_l.reshape(NCT, 128, D2)),
            "wv": _bf16(wv_l.reshape(NCT, 128, D2)),
            "bq": _f32(bq_f[hsl].reshape(D2)),
            "bk": _f32(bk_f[hsl].reshape(D2)),
            "bv": _bf16(bv_f[hsl].reshape(D2)),
            "wp": wp_host,
            "bp": bp_host,
            "w1": w1_host,
            "b1": b1_host,
            "w2": w2_host,
            "b2": b2_host,
            "cmw": cmw_host,
            "e16": e16_host,
        })
    return in_maps


def run(inputs, trace=False, trace_kwargs=None):
    nc = _get_program()
    in_maps = make_in_maps(inputs)
    res = run_bass_kernel_spmd(nc, in_maps, core_ids=list(range(N_CORES)),
                               trace=trace, **(trace_kwargs or {}))
    out = np.concatenate([res.results[j]["out"] for j in range(N_CORES)], axis=0)
    return out.reshape(1, T, C).astype(np.float32), res


def kernel(**inputs):
    out, _ = run(inputs)
    return out
